# revision 1
# baseline (speedup 1.0000x reference)
"""GCNII (8 layers, N=50000, E=800000) on 8 trn2 NeuronCores — v3.

Sharding: nodes partitioned into 8 contiguous ranges (6250/core); edges
partitioned by destination so each core owns the scatter-add for its node
range.

Per layer: segmented AllGather of dinv-scaled h (bf16, tight [*,64] rows
pair-packed into 256B gather rows, ping-pong buffers) into one DRAM
tensor PER SEGMENT, so each segment's arrival unblocks exactly the
gathers sourced from it (deep cross-layer pipelining). Each core
dma_gathers 256B pair-rows for its edges (tiles are segment- and
parity-pure), scatters them into PSUM via exact 0/1 one-hot matmuls
(bf16; one-hot tiles built in two batched DVE ops per chunk), then runs
the dense epilogue with the layer matrix folded as M = (1-b)I + b W.
GCN norm: dinv[src] folded into stored h, dinv[dst] applied as a
per-partition scale in the epilogue.
"""
import hashlib
import numpy as np
import ml_dtypes
import concourse.bass as bass
import concourse.mybir as mybir
from concourse import bacc, tile
from concourse.bass_utils import run_bass_kernel_spmd

mdt = mybir.dt
bf16 = ml_dtypes.bfloat16

N = 50000
E = 800000
FIN = 128
HID = 64
L = 8
ALPHA = 0.1
THETA = 0.5
NCORES = 8
NS = N // NCORES            # 6250 nodes per core
NW = (NS + 127) // 128      # 49 windows per core
NSPAD = NW * 128            # 6272
CW = 7                      # windows per chunk
CHUNKS = (NW + CW - 1) // CW       # 7 chunks
CHROW = [min(c * CW * 128, NS) for c in range(CHUNKS + 1)]
SEG_CH = [(0, 2), (2, 4), (4, 6), (6, 7)]   # allgather segments (chunks)


def _seg_structure(seg_ch):
    segrow = [(CHROW[a], CHROW[b]) for a, b in seg_ch]
    for r0, r1 in segrow:
        assert (r1 - r0) % 2 == 0
    # newpos-space bases of each segment (8 cores concatenated per segment)
    nbase = [8 * r0 for r0, _ in segrow]
    return segrow, nbase


def _preprocess(x, edge_index, w_in, b_in, conv_w, w_out, b_out,
                seg_ch=tuple(SEG_CH), src_seg=False):
    seg_ch = list(seg_ch)
    nseg = len(seg_ch)
    ngrp = (nseg * 2) if src_seg else 2
    segrow, nbase = _seg_structure(seg_ch)

    row = np.asarray(edge_index[0], dtype=np.int64)
    col = np.asarray(edge_index[1], dtype=np.int64)
    loops = np.arange(N, dtype=np.int64)
    row = np.concatenate([row, loops])
    col = np.concatenate([col, loops])
    deg = np.bincount(col, minlength=N).astype(np.float32)
    dinv = (1.0 / np.sqrt(deg)).astype(np.float32)

    # ---- seg-major renumbering: newpos[c*NS+r] = 8*a_s + c*len_s + (r-a_s)
    starts = np.array([s for s, _ in segrow], dtype=np.int64)
    ends = np.array([e for _, e in segrow], dtype=np.int64)
    r = np.arange(NS, dtype=np.int64)
    sid = np.searchsorted(ends, r, side="right")
    a, ln = starts[sid], (ends - starts)[sid]
    newpos = (8 * a + (r - a))[None, :] + np.arange(NCORES)[:, None] * ln[None, :]
    newpos = newpos.reshape(-1)
    nsrc_all = newpos[row].astype(np.int64)
    nbase_arr = np.array(nbase + [N], dtype=np.int64)
    seg_all = (np.searchsorted(nbase_arr, nsrc_all, side="right") - 1).astype(np.int32)

    # ---- per-core edges grouped by (window, src-seg, src-parity).
    # h stored tight bf16 [*,64]; viewed pairwise as 256B rows per segment;
    # tiles are (seg, parity)-pure: the matmul rhs slice 0:64/64:128 picks
    # the right node of the gathered pair.
    cores = []
    counts = np.zeros((NCORES, NW, ngrp), dtype=np.int64)
    for c in range(NCORES):
        m = (col >= c * NS) & (col < (c + 1) * NS)
        d = (col[m] - c * NS).astype(np.int32)
        s = nsrc_all[m]
        sg = seg_all[m]
        par = (s & 1).astype(np.int32)
        grp = (sg * 2 + par) if src_seg else par
        win = d >> 7
        key = win * ngrp + grp
        o = np.argsort(key, kind="stable")
        d, s, key = d[o], s[o], key[o]
        counts[c] = np.bincount(key, minlength=NW * ngrp).reshape(NW, ngrp)
        cores.append((d, s, key))

    Tg = -(-counts.max(axis=0) // 128)           # [NW, ngrp] tiles per group
    assert Tg.sum(axis=1).min() >= 1

    # global tile order: per chunk: seg-major, then window, then parity
    gidx = np.zeros((NW, ngrp), dtype=np.int64)
    g = 0
    CBASE = []
    SEGK = []                 # per chunk: per src-seg (or single) tile spans
    nsrcseg = nseg if src_seg else 1
    for sch in range(CHUNKS):
        CBASE.append(g)
        spans = []
        for sg in range(nsrcseg):
            g0 = g
            for w in range(sch * CW, min((sch + 1) * CW, NW)):
                for pc in range(2):
                    gidx[w, sg * 2 + pc] = g
                    g += Tg[w, sg * 2 + pc]
            spans.append((g0, g))
        SEGK.append(spans)
    T = g

    # ---- dense weights (shared across cores)
    w_in = np.asarray(w_in, np.float32)
    conv_w = np.asarray(conv_w, np.float32)
    w_out = np.asarray(w_out, np.float32)
    b_in = np.asarray(b_in, np.float32)
    b_out = np.asarray(b_out, np.float32)
    betas = np.log(THETA / np.arange(1, L + 1, dtype=np.float32) + 1.0)
    convT = np.concatenate(
        [((1.0 - betas[l]) * np.eye(HID, dtype=np.float32)
          + betas[l] * conv_w[l]).T for l in range(L)], axis=1)
    iota = np.tile(np.arange(128, dtype=np.float32), (128, 1))
    ident = np.eye(128, dtype=np.float32)
    consts = {
        "w_inT": np.ascontiguousarray(w_in.T).astype(bf16),
        "convT": np.ascontiguousarray(convT).astype(bf16),
        "w_outT": np.ascontiguousarray(w_out.T).astype(bf16),
        "b_in_rep": np.tile(b_in[None, :], (128, 1)),
        "b_out_rep": np.tile(b_out[None, :], (128, 1)),
        "iota_bf": iota.astype(bf16),
        "ident_bf": ident.astype(bf16),
    }

    x = np.asarray(x, np.float32)
    in_maps = []
    for c in range(NCORES):
        d, s, key = cores[c]
        gstart = np.searchsorted(key, np.arange(NW * ngrp))
        slot = np.arange(len(d)) - gstart[key]
        win = d >> 7
        grp = key - win * ngrp
        sg = grp >> 1
        tix = (gidx[win, grp] + (slot >> 7)).astype(np.int64)
        rix = (slot & 127).astype(np.int64)
        dstw8 = np.full((128, T), -1, dtype=np.int8)
        dstw8[rix, tix] = (d & 127).astype(np.int8)
        srcv = np.zeros((128, T), dtype=np.int32)
        if src_seg:
            srcv[rix, tix] = (s - nbase_arr[sg]) >> 1   # seg-local pair idx
        else:
            srcv[rix, tix] = s >> 1                     # global pair idx
        idx16 = (srcv.astype(np.int16).T.reshape(T, 8, 16)
                 .transpose(2, 0, 1).reshape(16, 8 * T))

        lidx = np.minimum(c * NS + np.arange(NSPAD), (c + 1) * NS - 1)
        dl = dinv[lidx].reshape(NW, 128).T
        xT = np.zeros((FIN, NSPAD), np.float32)
        xT[:, :NS] = x[c * NS: (c + 1) * NS].T
        in_maps.append(dict(
            consts,
            xT=np.ascontiguousarray(xT).astype(bf16),
            idx16=np.ascontiguousarray(idx16),
            dstw8=np.ascontiguousarray(dstw8),
            dinvs=np.ascontiguousarray(dl),
            dinv09=np.ascontiguousarray((1.0 - ALPHA) * dl),
        ))

    cfg = dict(Tg=Tg, gidx=gidx, T=T, CBASE=CBASE, SEGK=SEGK,
               seg_ch=seg_ch, src_seg=src_seg)
    return in_maps, cfg


def _build(cfg, reps=1, sim_single=False, ablate=(), nsplit=4,
           single_packet=False):
    ablate = set(ablate)
    Tg, gidx, T = cfg["Tg"], cfg["gidx"], cfg["T"]
    CBASE, SEGK, seg_ch = cfg["CBASE"], cfg["SEGK"], cfg["seg_ch"]
    src_seg = cfg["src_seg"]
    nseg = len(seg_ch)
    ngrp = (nseg * 2) if src_seg else 2
    segrow, nbase = _seg_structure(seg_ch)

    nc = bacc.Bacc(None, target_bir_lowering=False, num_devices=NCORES,
                   num_swdge_queues=4)

    xT_in = nc.dram_tensor("xT", [FIN, NSPAD], mdt.bfloat16, kind="ExternalInput")
    idx_in = nc.dram_tensor("idx16", [16, 8 * T], mdt.int16, kind="ExternalInput")
    dstw_in = nc.dram_tensor("dstw8", [128, T], mdt.int8, kind="ExternalInput")
    dinvs_in = nc.dram_tensor("dinvs", [128, NW], mdt.float32, kind="ExternalInput")
    dinv09_in = nc.dram_tensor("dinv09", [128, NW], mdt.float32, kind="ExternalInput")
    w_inT_in = nc.dram_tensor("w_inT", [FIN, HID], mdt.bfloat16, kind="ExternalInput")
    convT_in = nc.dram_tensor("convT", [HID, L * HID], mdt.bfloat16, kind="ExternalInput")
    w_outT_in = nc.dram_tensor("w_outT", [HID, HID], mdt.bfloat16, kind="ExternalInput")
    b_in_in = nc.dram_tensor("b_in_rep", [128, HID], mdt.float32, kind="ExternalInput")
    b_out_in = nc.dram_tensor("b_out_rep", [128, HID], mdt.float32, kind="ExternalInput")
    iota_in = nc.dram_tensor("iota_bf", [128, 128], mdt.bfloat16, kind="ExternalInput")
    ident_in = nc.dram_tensor("ident_bf", [128, 128], mdt.bfloat16, kind="ExternalInput")

    out_t = nc.dram_tensor("out", [NS, HID], mdt.bfloat16, kind="ExternalOutput")

    # per-segment bounce and (ping-pong x per-segment) gathered-h tensors
    bnc = [nc.dram_tensor(f"bounce{i}", [r1 - r0, HID], mdt.bfloat16)
           for i, (r0, r1) in enumerate(segrow)]
    if src_seg:
        hf = [[nc.dram_tensor(f"h_full{b}_{i}", [4 * (r1 - r0), FIN],
                              mdt.bfloat16, addr_space="Shared")
               for i, (r0, r1) in enumerate(segrow)] for b in range(2)]
    else:
        hf = [nc.dram_tensor(f"h_full{b}", [N // 2, FIN], mdt.bfloat16,
                             addr_space="Shared") for b in range(2)]
    seg_of_row = []
    for i, (r0, r1) in enumerate(segrow):
        seg_of_row += [i] * (r1 - r0)

    MAXK = max(CBASE[s + 1] - CBASE[s] if s + 1 < CHUNKS else T - CBASE[s]
               for s in range(CHUNKS))

    with tile.TileContext(nc) as tc, \
         tc.tile_pool(name="const", bufs=1) as cpool, \
         tc.tile_pool(name="gath", bufs=2) as gpool, \
         tc.tile_pool(name="oh", bufs=2) as ohpool, \
         tc.tile_pool(name="work", bufs=3) as wpool, \
         tc.tile_pool(name="ps_sc", bufs=3, space="PSUM") as psum_sc, \
         tc.tile_pool(name="ps_tr", bufs=2, space="PSUM") as psum_tr, \
         tc.tile_pool(name="ps_mm", bufs=2, space="PSUM") as psum_mm:

        # ---- persistent constants ----
        iota_t = cpool.tile([128, 128], mdt.bfloat16)
        nc.sync.dma_start(iota_t[:], iota_in[:])
        ident_t = cpool.tile([128, 128], mdt.bfloat16)
        nc.sync.dma_start(ident_t[:], ident_in[:])
        w_inT_t = cpool.tile([FIN, HID], mdt.bfloat16)
        nc.sync.dma_start(w_inT_t[:], w_inT_in[:])
        convT_t = cpool.tile([HID, L * HID], mdt.bfloat16)
        nc.sync.dma_start(convT_t[:], convT_in[:])
        w_outT_t = cpool.tile([HID, HID], mdt.bfloat16)
        nc.sync.dma_start(w_outT_t[:], w_outT_in[:])
        b_in_t = cpool.tile([128, HID], mdt.float32)
        nc.sync.dma_start(b_in_t[:], b_in_in[:])
        b_out_t = cpool.tile([128, HID], mdt.float32)
        nc.sync.dma_start(b_out_t[:], b_out_in[:])
        dinvs_t = cpool.tile([128, NW], mdt.float32)
        nc.sync.dma_start(dinvs_t[:], dinvs_in[:])
        dinv09_t = cpool.tile([128, NW], mdt.float32)
        nc.sync.dma_start(dinv09_t[:], dinv09_in[:])
        idx_t = cpool.tile([128, 8 * T], mdt.int16)
        for k in range(8):
            nc.sync.dma_start(idx_t[16 * k: 16 * (k + 1), :], idx_in[:])
        dstw8_t = cpool.tile([128, T], mdt.int8)
        nc.sync.dma_start(dstw8_t[:], dstw_in[:])
        xT_t = cpool.tile([FIN, NSPAD], mdt.bfloat16)
        nc.sync.dma_start(xT_t[:], xT_in[:])

        dstw_t = cpool.tile([128, T], mdt.bfloat16)
        nc.vector.tensor_copy(dstw_t[:], dstw8_t[:])

        h_sb = cpool.tile([128, NW * HID], mdt.bfloat16)
        x0s = cpool.tile([128, NW * HID], mdt.bfloat16)

        def store_h(w):
            nrows = min(NS - w * 128, 128)
            r0 = w * 128
            si = seg_of_row[r0]
            s0 = segrow[si][0]
            nc.sync.dma_start(
                bnc[si][r0 - s0: r0 - s0 + nrows, :],
                h_sb[:nrows, w * HID: (w + 1) * HID],
            )

        qctr = [0]

        for rep_i in range(reps):
            def allgather(seg, buf):
                r0, r1 = segrow[seg]
                if src_seg:
                    dst = hf[buf][seg][:, :]
                else:
                    dst = hf[buf][4 * r0: 4 * r1, :]
                if sim_single or "collective" in ablate:
                    nc.sync.dma_start(
                        dst[: (r1 - r0) // 2, :],
                        bnc[seg][:, :].rearrange("(a b) c -> a (b c)", b=2))
                else:
                    nc.gpsimd.collective_compute(
                        "AllGather", mybir.AluOpType.bypass,
                        replica_groups=[list(range(NCORES))],
                        ins=[bnc[seg][:, :]], outs=[dst],
                    )

            # ---- h0 = relu(x @ w_in.T + b_in); x0s = a*h0; h' = dinv*h0
            seg_ptr = 0
            for w in range(NW):
                ps = psum_mm.tile([128, HID], mdt.float32, tag="mm")
                nc.tensor.matmul(ps[:], xT_t[:, w * 128: (w + 1) * 128],
                                 w_inT_t[:], start=True, stop=True)
                u = wpool.tile([128, HID], mdt.float32, tag="u")
                nc.vector.tensor_tensor(u[:], ps[:], b_in_t[:], mybir.AluOpType.add)
                h0w = wpool.tile([128, HID], mdt.bfloat16, tag="h0")
                nc.scalar.activation(h0w[:], u[:], mybir.ActivationFunctionType.Relu)
                nc.vector.tensor_scalar_mul(x0s[:, w * HID: (w + 1) * HID],
                                            h0w[:], ALPHA)
                nc.vector.tensor_scalar(
                    h_sb[:, w * HID: (w + 1) * HID], h0w[:],
                    dinvs_t[:, w: w + 1], None, mybir.AluOpType.mult)
                store_h(w)
                while seg_ptr < nseg and (w + 1) * 128 >= segrow[seg_ptr][1]:
                    allgather(seg_ptr, 0)
                    seg_ptr += 1

            # ---- layers ----
            for l in range(L):
                sbuf = l % 2
                dbuf = (l + 1) % 2
                seg_ptr = 0
                for s in range(CHUNKS):
                    base = CBASE[s]
                    K = (CBASE[s + 1] if s + 1 < CHUNKS else T) - base
                    gt = gpool.tile([128, MAXK, FIN], mdt.bfloat16, tag="g")

                    for sg in range(len(SEGK[s])):
                        g0, g1 = SEGK[s][sg]
                        ntiles = g1 - g0
                        if "gather" in ablate or ntiles == 0:
                            continue
                        src_ap = (hf[sbuf][sg][:, :] if src_seg
                                  else hf[sbuf][:, :])
                        bnds = [g0 + (ntiles * i) // nsplit
                                for i in range(nsplit + 1)]
                        for i in range(nsplit):
                            a2, b2 = bnds[i], bnds[i + 1]
                            if a2 == b2:
                                continue
                            nidx = (b2 - a2) * 128
                            nc.gpsimd.dma_gather(
                                gt[:, a2 - base: b2 - base, :],
                                src_ap,
                                idx_t[:, 8 * a2: 8 * b2], nidx, nidx, FIN,
                                single_packet=single_packet,
                                queue_num=qctr[0] % 4,
                            )
                            qctr[0] += 1

                    if "onehot" not in ablate:
                        oh = ohpool.tile([128, MAXK, 128], mdt.bfloat16,
                                         tag="oh")
                        nc.vector.tensor_tensor(
                            oh[:, :K, :],
                            iota_t[:].unsqueeze(1).broadcast_to([128, K, 128]),
                            dstw_t[:, base: base + K].unsqueeze(2)
                                .broadcast_to([128, K, 128]),
                            mybir.AluOpType.is_equal)

                    for wi in range(CW):
                        w = s * CW + wi
                        if w >= NW:
                            break
                        ntiles = int(Tg[w].sum())
                        ps = psum_sc.tile([128, HID], mdt.float32, tag="sc")
                        k = 0
                        for grp in range(ngrp):
                            TT = int(Tg[w, grp])
                            g0 = int(gidx[w, grp])
                            pc = grp & 1
                            for t in range(TT):
                                gg = g0 + t
                                slot = gg - base
                                if "scatter_mm" not in ablate:
                                    oh_ap = (iota_t[:] if "onehot" in ablate
                                             else oh[:, slot, :])
                                    g_ap = (b_in_t[:].bitcast(mdt.bfloat16)[:, :HID]
                                            if "gather" in ablate
                                            else gt[:, slot, pc * HID: (pc + 1) * HID])
                                    nc.tensor.matmul(
                                        ps[:], oh_ap, g_ap,
                                        start=(k == 0), stop=(k == ntiles - 1))
                                k += 1
                        ps_ap = b_in_t[:] if "scatter_mm" in ablate else ps[:]
                        zw1 = wpool.tile([128, HID], mdt.bfloat16, tag="zw1")
                        nc.vector.tensor_scalar(
                            zw1[:], ps_ap, dinv09_t[:, w: w + 1], None,
                            mybir.AluOpType.mult)
                        zw = wpool.tile([128, HID], mdt.bfloat16, tag="zw")
                        nc.vector.tensor_tensor(
                            zw[:], zw1[:], x0s[:, w * HID: (w + 1) * HID],
                            mybir.AluOpType.add)
                        ztp = psum_tr.tile([HID, 128], mdt.bfloat16, tag="tr")
                        nc.tensor.transpose(ztp[:], zw[:], ident_t[:])
                        zt = wpool.tile([HID, 128], mdt.bfloat16, tag="zt")
                        nc.scalar.copy(zt[:], ztp[:])
                        ps2 = psum_mm.tile([128, HID], mdt.float32, tag="mm")
                        nc.tensor.matmul(ps2[:], zt[:],
                                         convT_t[:, l * HID: (l + 1) * HID],
                                         start=True, stop=True)
                        if l < L - 1:
                            nc.scalar.activation(
                                h_sb[:, w * HID: (w + 1) * HID], ps2[:],
                                mybir.ActivationFunctionType.Relu,
                                scale=dinvs_t[:, w: w + 1])
                            store_h(w)
                        else:
                            nc.scalar.activation(
                                h_sb[:, w * HID: (w + 1) * HID], ps2[:],
                                mybir.ActivationFunctionType.Relu)
                    if l < L - 1:
                        while (seg_ptr < nseg
                               and s + 1 >= seg_ch[seg_ptr][1]):
                            allgather(seg_ptr, dbuf)
                            seg_ptr += 1

            # ---- out = h @ w_out.T + b_out ----
            for w in range(NW):
                htp = psum_tr.tile([HID, 128], mdt.bfloat16, tag="tr")
                nc.tensor.transpose(htp[:], h_sb[:, w * HID: (w + 1) * HID],
                                    ident_t[:])
                ht = wpool.tile([HID, 128], mdt.bfloat16, tag="zt")
                nc.scalar.copy(ht[:], htp[:])
                ps3 = psum_mm.tile([128, HID], mdt.float32, tag="mm")
                nc.tensor.matmul(ps3[:], ht[:], w_outT_t[:], start=True, stop=True)
                ow = wpool.tile([128, HID], mdt.bfloat16, tag="ow")
                nc.vector.tensor_tensor(ow[:], ps3[:], b_out_t[:],
                                        mybir.AluOpType.add)
                nrows = min(NS - w * 128, 128)
                nc.sync.dma_start(out_t[w * 128: w * 128 + nrows, :],
                                  ow[:nrows, :])

    nc.finalize()
    return nc


_NC_CACHE = {}


def kernel(**inputs) -> np.ndarray:
    in_maps, cfg = _preprocess(
        inputs["x"], inputs["edge_index"], inputs["w_in"], inputs["b_in"],
        inputs["conv_w"], inputs["w_out"], inputs["b_out"],
    )
    key = hashlib.sha1(np.ascontiguousarray(inputs["edge_index"])).hexdigest()
    if key not in _NC_CACHE:
        _NC_CACHE[key] = _build(cfg)
    nc = _NC_CACHE[key]
    res = run_bass_kernel_spmd(nc, in_maps, list(range(NCORES)))
    out = np.concatenate(
        [res.results[c]["out"].astype(np.float32) for c in range(NCORES)], axis=0)
    return out

